# revision 20
# baseline (speedup 1.0000x reference)
"""Causal single-head attention on 8 trn2 NeuronCores.

Problem: x[4,2048,1024], Wq/Wk/Wv[1024,64] ->
  softmax(causal((x@Wq)@(x@Wk).T / 32)) @ (x@Wv)  -> [4,2048,64]

Sharding: 8 cores = 4 batches x 2 query-shards. Zigzag query split for
causal load balance: shard A handles query blocks {0,3} (of 512 rows),
shard B handles {1,2}. Each core redundantly computes K/V for the key
blocks it needs from a host-prepared chunk-major x[b].T (bf16).

SPMD uniformity: one program for all 8 cores. Per-core differences are
absorbed into data:
  - xt column-block permutation (A: [0,1,3,2], B: [1,0,2,3]) puts each
    core's diagonal (q==k) blocks at fixed program slots 0 and 4 of the
    pair list, so the diagonal structure is compile-time.
  - a tiny per-core bias input kills fully-masked (dummy) pairs via the
    exp() bias: exp(s - 1e5) == 0.

On-chip layout (scores kept transposed so softmax denominators and the
attention*V product are plain matmuls):
  qT[64,1024], kvT[128,2048] = (Wq|Wk|Wv)^T @ xt   (bf16 matmuls)
  scoresT[k,q] = kT_tile.T @ qT                     (per 128k x 512q tile)
  diagonal pairs only compute the live q-range [j*128:512] per k-tile j
  exp via ScalarE (bf16 out); diag 128-strips masked by triu mult (DVE)
  out_augT[66,512q] += v_aug_tile.T @ expT  where v_aug = [v | 1 | 1]
    gives the softmax denominator for free in row 64
  finalize: copy av psum -> sbuf, DMA transposed [66,512] block out;
  the host divides by the denominator row and transposes.
"""

import os
import sys

import ml_dtypes
import numpy as np

if "/opt/trn_rl_repo" not in sys.path and os.path.isdir("/opt/trn_rl_repo"):
    sys.path.insert(0, "/opt/trn_rl_repo")

import concourse.bacc as bacc
import concourse.mybir as mybir
import concourse.tile as tile
from concourse.bass_utils import run_bass_kernel_spmd

BF = ml_dtypes.bfloat16
B, S, E, H = 4, 2048, 1024, 64
BLK = 512  # kv/q block (4 blocks per sequence)
NCORES = 8
NE = E // 128  # 8 e-tiles
NCH = 4 * NE  # 32 xt chunks of [128, 512]
F32 = mybir.dt.float32
BF16 = mybir.dt.bfloat16
FEXP = mybir.ActivationFunctionType.Exp

# per-shard: query blocks and xt column-block permutation
QBLOCKS = {0: (0, 3), 1: (1, 2)}
PERM = {0: (0, 1, 3, 2), 1: (1, 0, 2, 3)}
# program-fixed pair list: (qslot, kslot); pairs 0 and 4 are diagonal
PAIRS = ((0, 0), (0, 1), (1, 0), (1, 1), (1, 2), (1, 3))
DIAG = (0, 4)
NEG = -1.0e5
VA = H + 2  # v | denom | denom-dup


def _build():
    nc = bacc.Bacc("TRN2", target_bir_lowering=False, debug=False, num_devices=NCORES)

    xt = nc.dram_tensor("xt", [NCH, 128, BLK], BF16, kind="ExternalInput").ap()
    # weights host-prearranged to SBUF layout: [p, (e h)], wkv then wq
    wqkv = nc.dram_tensor("wqkv", [128, NE * 3 * H], BF16, kind="ExternalInput").ap()
    bias2 = nc.dram_tensor("bias2", [128, 8], F32, kind="ExternalInput").ap()
    # triu | idmatb packed side by side
    tid = nc.dram_tensor("tid", [128, 256], BF16, kind="ExternalInput").ap()
    outT = nc.dram_tensor("outT", [VA, 2 * BLK], F32, kind="ExternalOutput").ap()

    xt_pcs = xt.rearrange("c p s -> p c s")

    with tile.TileContext(nc) as tc:
        with (
            tc.tile_pool(name="const", bufs=1) as cpool,
            tc.tile_pool(name="xt", bufs=4) as xtpool,
            tc.tile_pool(name="exp", bufs=6) as expool,
            tc.tile_pool(name="fin", bufs=2) as finpool,
            tc.tile_pool(name="kvps", bufs=1, space="PSUM") as kvps_pool,
            tc.tile_pool(name="qps", bufs=1, space="PSUM") as qps_pool,
            tc.tile_pool(name="vtps", bufs=1, space="PSUM") as vtps_pool,
            tc.tile_pool(name="stps", bufs=3, space="PSUM") as stps_pool,
            tc.tile_pool(name="avps", bufs=2, space="PSUM") as avps_pool,
        ):
            # ---- weights: split DMAs so the first matmuls unblock early ----
            wqkv_sb = cpool.tile([128, NE * 3 * H], BF16)
            wkv_sb = wqkv_sb[:, 0 : NE * 2 * H]
            wq_sb = wqkv_sb[:, NE * 2 * H :]

            xt_sb = [
                xtpool.tile([128, NE * BLK], BF16, name=f"xt_{s}", tag="xt")
                for s in range(4)
            ]

            def xts(s, e):
                return xt_sb[s][:, e * BLK : (e + 1) * BLK]

            def xt_dma(eng, s, e0, ne):
                eng.dma_start(
                    out=xt_sb[s].rearrange("p (e c) -> p e c", e=NE)[
                        :, e0 : e0 + ne, :
                    ],
                    in_=xt_pcs[:, s * NE + e0 : s * NE + e0 + ne, :],
                )

            # slot-0 chunks + weights are on the critical path: issue them
            # first on BOTH queues before any later-slot prefetch competes
            # for HBM bandwidth.
            tid_sb = cpool.tile([128, 256], BF16)
            triu_sb = tid_sb[:, 0:128]
            idmatb_sb = tid_sb[:, 128:256]
            bias2_sb = cpool.tile([128, 8], F32)

            # warm-up data for p-state ramp matmuls (no DMA dependency)
            warm_sb = cpool.tile([128, BLK + 128], BF16)
            nc.gpsimd.memset(warm_sb, 0.0)

            nc.sync.dma_start(
                out=wqkv_sb[:, 0 : 2 * 2 * H], in_=wqkv[:, 0 : 2 * 2 * H]
            )
            xt_dma(nc.sync, 0, 0, 1)
            xt_dma(nc.gpsimd, 0, 1, 1)
            nc.sync.dma_start(
                out=wqkv_sb[:, 2 * 2 * H : NE * 2 * H],
                in_=wqkv[:, 2 * 2 * H : NE * 2 * H],
            )
            xt_dma(nc.gpsimd, 0, 2, 2)
            xt_dma(nc.sync, 0, 4, 2)
            xt_dma(nc.gpsimd, 0, 6, 2)
            nc.sync.dma_start(out=wqkv_sb[:, NE * 2 * H :], in_=wqkv[:, NE * 2 * H :])
            nc.gpsimd.dma_start(out=tid_sb, in_=tid)
            nc.gpsimd.dma_start(out=bias2_sb, in_=bias2)
            xt_dma(nc.sync, 1, 0, 4)
            xt_dma(nc.gpsimd, 1, 4, 4)
            xt_dma(nc.sync, 2, 0, 4)
            xt_dma(nc.gpsimd, 2, 4, 4)
            xt_dma(nc.sync, 3, 0, 4)
            xt_dma(nc.gpsimd, 3, 4, 4)

            # persistent buffers
            kvT_sb = cpool.tile([128, S], BF16)  # rows 0:64 kT, 64:128 vT
            qT_sb = cpool.tile([64, 2 * BLK], BF16)
            vaug_sb = cpool.tile([128, 16 * VA], BF16)
            nc.gpsimd.memset(
                vaug_sb.rearrange("p (t c) -> p t c", t=16)[:, :, H : H + 2], 1.0
            )

            av_ps = [None, None]
            ex_t = {}

            def kv_chain(s, kv_ps, e0, e1):
                for e in range(e0, e1):
                    nc.tensor.matmul(
                        kv_ps,
                        wkv_sb[:, e * 128 : (e + 1) * 128],
                        xts(s, e),
                        start=(e == 0),
                        stop=(e == NE - 1),
                        skip_group_check=True,
                    )
                if e1 == NE:
                    nc.vector.tensor_copy(kvT_sb[:, s * BLK : (s + 1) * BLK], kv_ps)

            def p_q(s):
                q_ps = qps_pool.tile([64, BLK], F32, name=f"qps_{s}", tag="q")
                for e in range(NE):
                    nc.tensor.matmul(
                        q_ps,
                        wq_sb[:, e * H : (e + 1) * H],
                        xts(s, e),
                        start=(e == 0),
                        stop=(e == NE - 1),
                        skip_group_check=True,
                    )
                qs = s // 2
                nc.vector.tensor_copy(qT_sb[:, qs * BLK : (qs + 1) * BLK], q_ps)

            def p_tr(s):
                for j in range(4):
                    t = s * 4 + j
                    vt_ps = vtps_pool.tile([128, H], BF16, name=f"vt_{t}", tag="vt")
                    nc.tensor.transpose(
                        vt_ps,
                        kvT_sb[64:128, t * 128 : (t + 1) * 128],
                        idmatb_sb[64:128, 64:128],
                    )
                    nc.vector.tensor_copy(vaug_sb[:, t * VA : t * VA + H], vt_ps)

            def s_e(p, j):
                """score matmul + exp (+ triu mask) for pair p, k-tile j"""
                qslot, kslot = PAIRS[p]
                diag = p in DIAG
                lo = j * 128 if diag else 0
                st = stps_pool.tile([128, BLK], F32, name=f"st_{p}_{j}", tag="st")
                nc.tensor.matmul(
                    st[:, lo:],
                    kvT_sb[0:64, kslot * BLK + j * 128 : kslot * BLK + (j + 1) * 128],
                    qT_sb[0:64, qslot * BLK + lo : (qslot + 1) * BLK],
                    start=True,
                    stop=True,
                )
                ex = expool.tile([128, BLK], BF16, name=f"ex_{p}_{j}", tag="ex")
                bias = 0.0 if diag else bias2_sb[:, p : p + 1]
                nc.scalar.activation(ex[:, lo:], st[:, lo:], FEXP, bias=bias)
                if diag:
                    nc.vector.tensor_tensor(
                        ex[:, j * 128 : (j + 1) * 128],
                        ex[:, j * 128 : (j + 1) * 128],
                        triu_sb,
                        mybir.AluOpType.mult,
                    )
                ex_t[(p, j)] = ex

            def a_(p, j, start=False, stop=False):
                qslot, kslot = PAIRS[p]
                diag = p in DIAG
                lo = j * 128 if diag else 0
                if av_ps[qslot] is None:
                    av_ps[qslot] = avps_pool.tile(
                        [VA, BLK], F32, name=f"av_{qslot}", tag="av"
                    )
                t = kslot * 4 + j
                nc.tensor.matmul(
                    av_ps[qslot][:, lo:],
                    vaug_sb[:, t * VA : (t + 1) * VA],
                    ex_t[(p, j)][:, lo:],
                    start=start,
                    stop=stop,
                    skip_group_check=True,
                )

            def fin(qs):
                # split the copy and the store across engines/queues so the
                # post-compute tail is short
                oT = finpool.tile([VA, BLK], F32, name=f"oT_{qs}", tag="oT")
                nc.vector.tensor_copy(oT[:, 0:256], av_ps[qs][:, 0:256])
                nc.scalar.activation(
                    oT[:, 256:512],
                    av_ps[qs][:, 256:512],
                    mybir.ActivationFunctionType.Copy,
                )
                c0 = qs * BLK
                nc.sync.dma_start(out=outT[:, c0 : c0 + 256], in_=oT[:, 0:256])
                nc.gpsimd.dma_start(
                    out=outT[:, c0 + 256 : c0 + 512], in_=oT[:, 256:512]
                )

            # ---------------- PE-stream schedule ----------------
            kv0 = kvps_pool.tile([128, BLK], F32, name="kvps_0", tag="kv")
            # warm-up matmuls: ramp the PE p-state while the first xt chunks
            # stream in; results are overwritten by the real chain (start=True)
            for _ in range(12):
                nc.tensor.matmul(
                    kv0,
                    warm_sb[:, BLK:],
                    warm_sb[:, 0:BLK],
                    start=True,
                    stop=True,
                    skip_group_check=True,
                )
            kv_chain(0, kv0, 0, NE)
            p_q(0)
            p_tr(0)
            s_e(0, 0); s_e(0, 1); s_e(0, 2)
            a_(0, 0, start=True)
            s_e(0, 3)
            kv1 = kvps_pool.tile([128, BLK], F32, name="kvps_1", tag="kv")
            a_(0, 1); kv_chain(1, kv1, 0, 2)
            a_(0, 2); kv_chain(1, kv1, 2, 4)
            a_(0, 3); kv_chain(1, kv1, 4, NE)
            s_e(1, 0); s_e(1, 1); s_e(1, 2)
            p_tr(1)
            a_(1, 0); s_e(1, 3); a_(1, 1); a_(1, 2); a_(1, 3, stop=True)
            fin(0)
            kv2 = kvps_pool.tile([128, BLK], F32, name="kvps_2", tag="kv")
            kv_chain(2, kv2, 0, NE)
            p_q(2)
            s_e(2, 0); s_e(2, 1); s_e(2, 2)
            p_tr(2)
            a_(2, 0, start=True); s_e(2, 3); a_(2, 1); a_(2, 2); a_(2, 3)
            s_e(4, 0); s_e(4, 1)
            kv3 = kvps_pool.tile([128, BLK], F32, name="kvps_3", tag="kv")
            kv_chain(3, kv3, 0, 4)
            a_(4, 0); s_e(4, 2); a_(4, 1); s_e(4, 3); a_(4, 2); a_(4, 3)
            kv_chain(3, kv3, 4, NE)
            s_e(3, 0); s_e(3, 1); s_e(3, 2)
            p_tr(3)
            a_(3, 0); s_e(3, 3); a_(3, 1); a_(3, 2); a_(3, 3)
            s_e(5, 0); s_e(5, 1); s_e(5, 2)
            a_(5, 0); s_e(5, 3); a_(5, 1); a_(5, 2); a_(5, 3, stop=True)
            fin(1)

    nc.compile()
    return nc


_NC_CACHE = None
RUN_KWARGS = {}  # test harness may set {"trace": True}
LAST_RESULTS = None  # BassKernelResults of the most recent run


def kernel(x, Wq, Wk, Wv):
    global _NC_CACHE, LAST_RESULTS
    x = np.asarray(x, dtype=np.float32)
    Wq = np.asarray(Wq, dtype=np.float32)
    Wk = np.asarray(Wk, dtype=np.float32)
    Wv = np.asarray(Wv, dtype=np.float32)

    def to_sb(w):  # [E, h] -> [128, NE*h] with e-tiles side by side
        h = w.shape[1]
        return np.ascontiguousarray(
            w.reshape(NE, 128, h).transpose(1, 0, 2).reshape(128, NE * h)
        )

    wq_s = to_sb(Wq / np.float32(E**0.5))
    wkv = to_sb(np.concatenate([Wk, Wv], axis=1))
    wqkv = np.ascontiguousarray(np.concatenate([wkv, wq_s], axis=1)).astype(BF)
    triu = np.triu(np.ones((128, 128), dtype=np.float32))
    tid = np.concatenate([triu, np.eye(128, dtype=np.float32)], axis=1).astype(BF)

    in_maps = []
    for core in range(NCORES):
        b, shard = divmod(core, 2)
        perm = PERM[shard]
        # chunk-major xt: chunk[s*NE + e] = x[b].T[e-tile, block perm[s]]
        xtp = x[b].T.reshape(NE, 128, 4, BLK)  # [e, p, block, col]
        xt = np.ascontiguousarray(
            xtp.transpose(2, 0, 1, 3)[list(perm)].reshape(NCH, 128, BLK)
        ).astype(BF)
        qb = QBLOCKS[shard]
        bias2 = np.zeros((128, 8), dtype=np.float32)
        for p, (qslot, kslot) in enumerate(PAIRS):
            if perm[kslot] > qb[qslot]:  # key block entirely in the future
                bias2[:, p] = NEG
        in_maps.append(dict(xt=xt, wqkv=wqkv, bias2=bias2, tid=tid))

    if _NC_CACHE is None:
        _NC_CACHE = _build()
    res = run_bass_kernel_spmd(
        _NC_CACHE, in_maps, core_ids=list(range(NCORES)), **RUN_KWARGS
    )
    LAST_RESULTS = res

    out = np.empty((B, S, H), dtype=np.float32)
    for core in range(NCORES):
        b, shard = divmod(core, 2)
        o = res.results[core]["outT"]  # [VA, 2*BLK]
        for qs, blk in enumerate(QBLOCKS[shard]):
            sub = o[:, qs * BLK : (qs + 1) * BLK]
            out[b, blk * BLK : (blk + 1) * BLK, :] = (sub[0:64] / sub[64:65]).T
    return out
